# revision 21
# baseline (speedup 1.0000x reference)
"""Trainium2 Bass kernel for nn_CellEncoder (2-layer GraphSAGE, mean aggregation).

Strategy (8 NeuronCores, SPMD, node-partitioned):
  - Core c owns nodes [c*npc, (c+1)*npc).  Aggregation is linear, so the
    dense transform is applied FIRST: z = h @ W_l.T reduces gather width
    from in_dim (1000) to emb (128) per edge.
  - All PE-path data is bf16 (x, weights, z tables, gathered rows, one-hot
    S): bf16 matmuls run 4x faster than fp32 on TRN2 PE and halve the
    AllGather + gather traffic.  PSUM accumulation stays fp32.
  - Per layer: each core computes z for its own nodes, contributes two
    half-slabs to two AllGathers forming table_lo/table_hi (each
    NC*npc/2 rows < 32768 so rows are addressable by int16 dma_gather
    indices).  The lo AllGather is issued mid-loop as soon as the lo
    tiles are produced, overlapping the collective with compute.
  - Edges are grouped by dst tile (128 dsts); each tile's edges are packed
    into chunks of 128 slots (lo-table chunks then hi-table chunks).
    dma_gather (round-robin over all 4 SWDGE queues) pulls the slot rows
    into SBUF; one-hot matrices S[e,d] = (dst(e)==d) are generated on DVE
    in ONE batched is_equal per st-group (3-d broadcast APs, bf16 2x
    mode) and drive the PE accumulation aggT[f,d] += G_chunk.T @ S_chunk
    in PSUM.  Lo-chunk matmuls are issued before hi-chunk matmuls so the
    PE starts as soon as the lo gather lands.
  - Epilogue is feature-major: x = aggT*inv + rb; elu via
    relu(x) [Scalar engine] + exp(min(x,0)) [Scalar] - 1, bf16 on DVE.
  - Output written feature-major bf16 [128, NPAD]; host transposes/trims.

kernel(**inputs) takes FULL inputs, shards internally, runs one NEFF on
cores 0-7 via bass_utils.run_bass_kernel_spmd, returns the full output.
"""
import os
import sys

import numpy as np

for _p in ("/opt/trn_rl_repo", "/root/.axon_site/_ro/trn_rl_repo"):
    if os.path.isdir(_p) and _p not in sys.path:
        sys.path.append(_p)

import ml_dtypes

import concourse.bass as bass
import concourse.bacc as bacc
import concourse.mybir as mybir
import concourse.tile as tile
from concourse import bass_utils

P = 128
F32 = mybir.dt.float32
BF16 = mybir.dt.bfloat16
AF = mybir.ActivationFunctionType
ALU = mybir.AluOpType

# SWDGE descriptor-ring sizing: ring holds scratch//16 descriptors; one
# dma_gather must fit in its queue's ring.
SCRATCH = 49152
GMAX = 22  # chunks (2816 idxs) per dma_gather; ring holds 3072


def build_meta(N, NC, dst, src, tiles_per_st):
    """Static chunk structure (shared across cores; max-over-core sizes) and
    per-core gather-index / dst-id slabs."""
    npc = N // NC
    half = npc // 2
    TPC = (npc + P - 1) // P
    NPAD = TPC * P
    NST = (TPC + tiles_per_st - 1) // tiles_per_st

    c = dst // npc
    d = (dst - c * npc).astype(np.int64)
    t = d // P
    did = d % P
    sc = src // npc
    sp = src - sc * npc
    tb = (sp >= half).astype(np.int64)
    row = sc * half + np.where(tb == 0, sp, sp - half)
    assert row.max() < 32768

    nlohi = np.zeros((NC, TPC, 2), np.int64)
    np.add.at(nlohi, (c, t, tb), 1)
    KL = np.maximum(1, (nlohi[:, :, 0].max(axis=0) + P - 1) // P)
    KH = ((nlohi[:, :, 1].max(axis=0) + P - 1) // P).astype(np.int64)

    Ktot = KL + KH
    chunk_base = np.concatenate([[0], np.cumsum(Ktot)])
    NCHUNK = int(chunk_base[-1])

    st_tiles = [list(range(s * tiles_per_st, min((s + 1) * tiles_per_st, TPC)))
                for s in range(NST)]
    GL = [int(sum(KL[tt] for tt in ts)) for ts in st_tiles]
    GH = [int(sum(KH[tt] for tt in ts)) for ts in st_tiles]

    idx_off_lo, idx_off_hi = [], []
    off = 0
    for s in range(NST):
        idx_off_lo.append(off); off += GL[s] * P // 16
        idx_off_hi.append(off); off += GH[s] * P // 16
    NIDX16 = off

    idx_slab = np.zeros((NC, P, NIDX16), np.int16)
    dstid_slab = np.full((NC, P, NCHUNK), -1.0, np.float32)
    cnt = np.zeros((NC, NPAD), np.int64)

    order = np.lexsort((tb, t, c))
    co, to, tbo = c[order], t[order], tb[order]
    rowo, dido, do_ = row[order], did[order], d[order]
    np.add.at(cnt, (co, do_), 1)

    key = (co * TPC + to) * 2 + tbo
    bounds = np.concatenate([[0], np.nonzero(np.diff(key))[0] + 1, [len(key)]])
    gval_lo = [np.zeros((NC, GL[s] * P), np.int16) for s in range(NST)]
    gval_hi = [np.zeros((NC, GH[s] * P), np.int16) for s in range(NST)]

    lo_base = np.zeros(TPC, np.int64)
    hi_base = np.zeros(TPC, np.int64)
    for s, ts in enumerate(st_tiles):
        accl = acch = 0
        for tt in ts:
            lo_base[tt] = accl; accl += KL[tt] * P
            hi_base[tt] = acch; acch += KH[tt] * P

    for bi in range(len(bounds) - 1):
        lo_, hi_ = int(bounds[bi]), int(bounds[bi + 1])
        if lo_ == hi_:
            continue
        cc, tt, bb = int(co[lo_]), int(to[lo_]), int(tbo[lo_])
        n = hi_ - lo_
        s = tt // tiles_per_st
        if bb == 0:
            base = int(lo_base[tt])
            gval_lo[s][cc, base:base + n] = rowo[lo_:hi_]
            ch0 = int(chunk_base[tt])
        else:
            base = int(hi_base[tt])
            gval_hi[s][cc, base:base + n] = rowo[lo_:hi_]
            ch0 = int(chunk_base[tt]) + int(KL[tt])
        # base is a multiple of P: slot partition (base+i)%P == i%P and
        # gather block base//P + i//P lines up with tile chunk ch0 + i//P.
        local = np.arange(n)
        dstid_slab[cc, local % P, ch0 + local // P] = dido[lo_:hi_]

    for s in range(NST):
        for cc in range(NC):
            for vals, o in ((gval_lo[s][cc], idx_off_lo[s]),
                            (gval_hi[s][cc], idx_off_hi[s])):
                n = len(vals)
                if n == 0:
                    continue
                w = vals.reshape(n // 16, 16).T
                idx_slab[cc, :, o:o + n // 16] = np.tile(w, (8, 1))

    inv = (1.0 / np.maximum(cnt, 1)).astype(np.float32)

    return dict(
        npc=npc, half=half, TPC=TPC, NPAD=NPAD, NST=NST, st_tiles=st_tiles,
        KL=[int(v) for v in KL], KH=[int(v) for v in KH],
        chunk_base=[int(v) for v in chunk_base], NCHUNK=NCHUNK,
        GL=GL, GH=GH, idx_off_lo=idx_off_lo, idx_off_hi=idx_off_hi,
        NIDX16=NIDX16, idx_slab=idx_slab, dstid_slab=dstid_slab, inv=inv,
    )


# ---------------------------------------------------------------------------
# device kernel builder
# ---------------------------------------------------------------------------

def build_kernel(meta, in_dim, NC):
    npc, half = meta["npc"], meta["half"]
    TPC, NPAD, NST = meta["TPC"], meta["NPAD"], meta["NST"]
    NCHUNK, NIDX16 = meta["NCHUNK"], meta["NIDX16"]
    KL, KH, chunk_base = meta["KL"], meta["KH"], meta["chunk_base"]
    GC = (in_dim + P - 1) // P
    GPAD = GC * P
    XTOT = sum(GC * len(ts) * P for ts in meta["st_tiles"])
    gq = [0]  # gather queue round-robin over 0..3
    # st index after which the lo half (rows < half) is fully produced
    lo_done_st = next(s for s, ts in enumerate(meta["st_tiles"])
                      if (ts[-1] + 1) * P >= half)

    nc = bacc.Bacc("TRN2", target_bir_lowering=False, debug=False,
                   enable_asserts=False, num_devices=NC,
                   dynamic_dma_scratch_size=SCRATCH, num_swdge_queues=4)

    x_d = nc.dram_tensor("x_sw", [P, XTOT], BF16, kind="ExternalInput").ap()
    w0l_d = nc.dram_tensor("W0lT", [GPAD, P], BF16, kind="ExternalInput").ap()
    w0r_d = nc.dram_tensor("W0rT", [GPAD, P], BF16, kind="ExternalInput").ap()
    w1l_d = nc.dram_tensor("W1lT", [P, P], BF16, kind="ExternalInput").ap()
    w1r_d = nc.dram_tensor("W1rT", [P, P], BF16, kind="ExternalInput").ap()
    b0_d = nc.dram_tensor("b0col", [P, 1], F32, kind="ExternalInput").ap()
    b1_d = nc.dram_tensor("b1col", [P, 1], F32, kind="ExternalInput").ap()
    inv_d = nc.dram_tensor("invt", [P, NPAD], BF16, kind="ExternalInput").ap()
    idx_d = nc.dram_tensor("idx16", [P, NIDX16], mybir.dt.int16,
                           kind="ExternalInput").ap()
    iota_d = nc.dram_tensor("iota", [P, P], BF16, kind="ExternalInput").ap()
    dst_d = nc.dram_tensor("dstid", [P, NCHUNK], BF16, kind="ExternalInput").ap()
    out_d = nc.dram_tensor("outT", [P, NPAD], BF16, kind="ExternalOutput").ap()

    with tile.TileContext(nc, num_cores=NC) as tc:
        with (
            tc.tile_pool(name="const", bufs=1) as cpool,
            tc.tile_pool(name="slab", bufs=1) as slab,
            tc.tile_pool(name="xp", bufs=2) as xpool,
            tc.tile_pool(name="gat", bufs=6) as gpool,
            tc.tile_pool(name="sp", bufs=2) as spool,
            tc.tile_pool(name="zp", bufs=3) as zpool,
            tc.tile_pool(name="ep", bufs=2) as epool,
            tc.tile_pool(name="pz", bufs=2, space="PSUM") as pz,
            tc.tile_pool(name="pr", bufs=2, space="PSUM") as pr,
            tc.tile_pool(name="pa", bufs=2, space="PSUM") as pa,
            tc.tile_pool(name="dram", bufs=1, space="DRAM") as dram,
        ):
            # ---- constants ----
            w0l_sb = cpool.tile([P, GC * P], BF16)
            w0r_sb = cpool.tile([P, GC * P], BF16)
            for gc in range(GC):
                nc.sync.dma_start(out=w0l_sb[:, gc * P:(gc + 1) * P],
                                  in_=w0l_d[gc * P:(gc + 1) * P, :])
                nc.sync.dma_start(out=w0r_sb[:, gc * P:(gc + 1) * P],
                                  in_=w0r_d[gc * P:(gc + 1) * P, :])
            w1l_sb = cpool.tile([P, P], BF16)
            nc.sync.dma_start(out=w1l_sb[:], in_=w1l_d[:])
            w1r_sb = cpool.tile([P, P], BF16)
            nc.sync.dma_start(out=w1r_sb[:], in_=w1r_d[:])
            b0_sb = cpool.tile([P, 1], F32)
            nc.sync.dma_start(out=b0_sb[:], in_=b0_d[:])
            b1_sb = cpool.tile([P, 1], F32)
            nc.sync.dma_start(out=b1_sb[:], in_=b1_d[:])
            zero_sb = cpool.tile([P, 1], BF16)
            nc.vector.memset(zero_sb[:], 0.0)
            mone_sb = cpool.tile([P, 1], BF16)
            nc.vector.memset(mone_sb[:], -1.0)
            iota_sb = cpool.tile([P, P], BF16)
            nc.sync.dma_start(out=iota_sb[:], in_=iota_d[:])
            dst_sb = cpool.tile([P, NCHUNK], BF16)
            nc.sync.dma_start(out=dst_sb[:], in_=dst_d[:])
            inv_sb = cpool.tile([P, NPAD], BF16)
            nc.sync.dma_start(out=inv_sb[:], in_=inv_d[:])
            idx_sb = cpool.tile([P, NIDX16], mybir.dt.int16)
            nc.sync.dma_start(out=idx_sb[:], in_=idx_d[:])
            GW = max(meta["GL"][s] + meta["GH"][s] for s in range(NST))
            GHALF = max(meta["GL"] + meta["GH"])

            rb0_sb = slab.tile([P, NPAD], BF16)
            rb1_sb = slab.tile([P, NPAD], BF16)

            # ---- collective buffers ----
            def cc_pair(nm):
                i_lo = dram.tile([half, P], BF16, name=f"cci_lo{nm}")
                i_hi = dram.tile([half, P], BF16, name=f"cci_hi{nm}")
                o_lo = dram.tile([NC * half, P], BF16, addr_space="Shared",
                                 name=f"cco_lo{nm}")
                o_hi = dram.tile([NC * half, P], BF16, addr_space="Shared",
                                 name=f"cco_hi{nm}")
                return i_lo, i_hi, o_lo, o_hi

            cc0 = cc_pair("0")
            cc1 = cc_pair("1")
            rg = [list(range(NC))]

            def z_to_cc(z_sb, tt, cc):
                # scalar-engine HWDGE ring: keeps these cast-gated writes off
                # the sync ring so x loads / idx preloads stream freely
                r0, r1 = tt * P, min(tt * P + P, npc)
                for lo_s, hi_s, tgt, base in (
                        (r0, min(r1, half), cc[0], 0),
                        (max(r0, half), r1, cc[1], half)):
                    if hi_s > lo_s:
                        nc.scalar.dma_start(
                            out=tgt[lo_s - base:hi_s - base, :],
                            in_=z_sb[lo_s - r0:hi_s - r0, :])

            def ag(cc, which):
                nc.gpsimd.collective_compute(
                    "AllGather", ALU.bypass, replica_groups=rg,
                    ins=[cc[which][:].opt()], outs=[cc[which + 2][:].opt()])

            # ---- phase A: z0 (node-major) + rb0T (feature-major) ----
            xoff = 0
            for s, ts in enumerate(meta["st_tiles"]):
                w = len(ts) * P
                c0 = ts[0] * P
                xg = xpool.tile([P, GC * w], BF16, tag="xg",
                                padded_shape=[P, GC * 2 * P])
                nc.sync.dma_start(out=xg[:], in_=x_d[:, xoff:xoff + GC * w])
                xoff += GC * w
                r0ps = pr.tile([P, w], F32, tag="rps", padded_shape=[P, 2 * P])
                for gc in range(GC):
                    nc.tensor.matmul(out=r0ps[:],
                                     lhsT=w0r_sb[:, gc * P:(gc + 1) * P],
                                     rhs=xg[:, gc * w:(gc + 1) * w],
                                     start=(gc == 0), stop=(gc == GC - 1))
                nc.vector.tensor_tensor(out=rb0_sb[:, c0:c0 + w], in0=r0ps[:],
                                        in1=b0_sb[:, :1].to_broadcast([P, w]),
                                        op=ALU.add)
                for ti, tt in enumerate(ts):
                    z0ps = pz.tile([P, P], F32, tag="zps")
                    for gc in range(GC):
                        nc.tensor.matmul(
                            out=z0ps[:],
                            lhsT=xg[:, gc * w + ti * P:gc * w + (ti + 1) * P],
                            rhs=w0l_sb[:, gc * P:(gc + 1) * P],
                            start=(gc == 0), stop=(gc == GC - 1))
                    z0sb = zpool.tile([P, P], BF16, tag="zsb")
                    nc.vector.tensor_copy(out=z0sb[:], in_=z0ps[:])
                    z_to_cc(z0sb, tt, cc0)
                if s == lo_done_st:
                    ag(cc0, 0)
            ag(cc0, 1)

            def gather_split(table, nch, idx_sb, idx_off, tag, bufs=3):
                """One or more dma_gathers (<= GMAX chunks each) into one
                SBUF tile [P, nch*P]."""
                if nch == 0:
                    return None
                g = gpool.tile([P, nch * P], BF16, tag=tag, bufs=bufs,
                               padded_shape=[P, GHALF * P])
                done = 0
                while done < nch:
                    n = min(GMAX, nch - done)
                    gq[0] = (gq[0] + 1) % 4
                    nc.gpsimd.dma_gather(
                        out_ap=g[:, done * P:(done + n) * P]
                        .rearrange("p (k e) -> p k e", e=P),
                        in_ap=table[:],
                        idxs_ap=idx_sb[:, idx_off + done * 8:
                                       idx_off + (done + n) * 8],
                        num_idxs=n * P, num_idxs_reg=n * P, elem_size=P,
                        single_packet=(n * P <= 1024), queue_num=gq[0])
                    done += n
                return g

            # ---- aggregation + epilogue (shared for both layers) ----
            # ghi lags glo by LAG st-groups so AG-hi waiters never exhaust the
            # GpSimd wait queue (which would stall glo issue at layer start)
            LAG = 5

            def aggregate(s, ts, glo, ghi, rb_slab, out_cb, post_cb=None):
                w = len(ts) * P
                c0 = ts[0] * P
                GLs, GHs = meta["GL"][s], meta["GH"][s]
                nch_st = GLs + GHs
                # batched one-hot S for all chunks of this st-group:
                # S[p, j, d] = (dstid[p, chunk j] == d), bf16
                cb0 = chunk_base[ts[0]]
                s_all = spool.tile([P, nch_st * P], BF16, tag="ssb",
                                   padded_shape=[P, GW * P])
                nc.vector.tensor_tensor(
                    out=s_all[:].rearrange("p (n d) -> p n d", d=P),
                    in0=dst_sb[:, cb0:cb0 + nch_st].to_broadcast([P, nch_st, P]),
                    in1=iota_sb[:].rearrange("p (one d) -> p one d", one=1)
                    .to_broadcast([P, nch_st, P]),
                    op=ALU.is_equal)

                def s_col(cg):
                    j = cg - cb0
                    return s_all[:, j * P:(j + 1) * P]

                # separate PSUM tiles per dst tile so lo-first ordering keeps
                # at most one open accumulation group per PSUM bank
                aggps = [pa.tile([P, P], F32, tag=f"aggps{ti}",
                                 name=f"aggps{ti}") for ti in range(len(ts))]
                # lo chunks first (start accumulation), then hi chunks (stop)
                lo_blk = 0
                for ti, tt in enumerate(ts):
                    for j in range(KL[tt]):
                        g_ap = glo[:, (lo_blk + j) * P:(lo_blk + j + 1) * P]
                        nc.tensor.matmul(out=aggps[ti][:],
                                         lhsT=g_ap, rhs=s_col(chunk_base[tt] + j),
                                         start=(j == 0), stop=False)
                    lo_blk += KL[tt]
                hi_blk = 0
                for ti, tt in enumerate(ts):
                    for j in range(KH[tt]):
                        g_ap = ghi[:, (hi_blk + j) * P:(hi_blk + j + 1) * P]
                        nc.tensor.matmul(out=aggps[ti][:],
                                         lhsT=g_ap,
                                         rhs=s_col(chunk_base[tt] + KL[tt] + j),
                                         start=False, stop=(j == KH[tt] - 1))
                    hi_blk += KH[tt]
                # epilogue: x = aggT*inv + rb ; h = relu(x) + exp(min(x,0)) - 1
                x2 = epool.tile([P, w], BF16, tag="x2", padded_shape=[P, 2 * P])
                for ti, tt in enumerate(ts):
                    nc.vector.tensor_tensor(
                        out=x2[:, ti * P:(ti + 1) * P], in0=aggps[ti][:],
                        in1=inv_sb[:, tt * P:(tt + 1) * P], op=ALU.mult)
                x3 = epool.tile([P, w], BF16, tag="x3", padded_shape=[P, 2 * P])
                nc.vector.tensor_tensor(out=x3[:], in0=x2[:],
                                        in1=rb_slab[:, c0:c0 + w], op=ALU.add)
                xm = epool.tile([P, w], BF16, tag="xm", padded_shape=[P, 2 * P])
                nc.scalar.activation(out=xm[:], in_=x3[:], func=AF.Relu)
                xc = epool.tile([P, w], BF16, tag="xc", padded_shape=[P, 2 * P])
                nc.vector.tensor_tensor(out=xc[:], in0=x3[:],
                                        in1=zero_sb[:, :1].to_broadcast([P, w]),
                                        op=ALU.min)
                xe = epool.tile([P, w], BF16, tag="xe", padded_shape=[P, 2 * P])
                nc.scalar.activation(out=xe[:], in_=xc[:], func=AF.Exp)
                h = epool.tile([P, w], BF16, tag="h", padded_shape=[P, 2 * P])
                nc.vector.scalar_tensor_tensor(out=h[:], in0=xm[:], scalar=-1.0,
                                               in1=xe[:], op0=ALU.add,
                                               op1=ALU.add)
                out_cb(s, ts, w, c0, h)
                if post_cb is not None:
                    post_cb()

            # ---- phase B+C: layer-0 aggregate -> h1T -> z1/rb1T ----
            def l0_out(s, ts, w, c0, h):
                for ti, tt in enumerate(ts):
                    z1ps = pz.tile([P, P], F32, tag="zps")
                    nc.tensor.matmul(out=z1ps[:],
                                     lhsT=h[:, ti * P:(ti + 1) * P],
                                     rhs=w1l_sb[:], start=True, stop=True)
                    z1sb = zpool.tile([P, P], BF16, tag="zsb")
                    nc.vector.tensor_copy(out=z1sb[:], in_=z1ps[:])
                    z_to_cc(z1sb, tt, cc1)
                r1ps = pr.tile([P, w], F32, tag="rps", padded_shape=[P, 2 * P])
                nc.tensor.matmul(out=r1ps[:], lhsT=w1r_sb[:], rhs=h[:],
                                 start=True, stop=True)
                nc.vector.tensor_tensor(out=rb1_sb[:, c0:c0 + w], in0=r1ps[:],
                                        in1=b1_sb[:, :1].to_broadcast([P, w]),
                                        op=ALU.add)

            def run_layer(tables, rb_slab, out_cb, post_idx=None, post_cb=None):
                # issue gathers in groups of 4 (2 st-groups x lo/hi) so each
                # sequencer stall quantum feeds all 4 SWDGE queue pairs
                glos, ghis = {}, {}
                for s in range(0, NST + LAG + 1, 2):
                    for j in (s, s + 1):
                        if j < NST:
                            glos[j] = gather_split(
                                tables[0], meta["GL"][j], idx_sb,
                                meta["idx_off_lo"][j], "glo", bufs=LAG + 6)
                    for j in (s, s + 1):
                        t = j - LAG
                        if 0 <= t < NST:
                            o_lo = meta["idx_off_lo"][t]
                            ghis[t] = gather_split(
                                tables[1], meta["GH"][t], idx_sb,
                                o_lo + meta["GL"][t] * 8, "ghi", bufs=7)
                    for j in (s, s + 1):
                        t = j - LAG
                        if 0 <= t < NST:
                            aggregate(t, meta["st_tiles"][t], glos.pop(t),
                                      ghis.pop(t), rb_slab, out_cb,
                                      post_cb=post_cb if t == post_idx
                                      else None)

            run_layer((cc0[2], cc0[3]), rb0_sb, l0_out,
                      post_idx=lo_done_st, post_cb=lambda: ag(cc1, 0))
            ag(cc1, 1)

            # ---- phase D: layer-1 aggregate -> output ----
            def l1_out(s, ts, w, c0, h):
                nc.sync.dma_start(out=out_d[:, c0:c0 + w], in_=h[:])

            run_layer((cc1[2], cc1[3]), rb1_sb, l1_out)

    nc.compile()
    return nc


# ---------------------------------------------------------------------------
# entry point
# ---------------------------------------------------------------------------

def _prepare(x, knn_edge_index, W_l0, b_l0, W_r0, W_l1, b_l1, W_r1,
             NC=8, tiles_per_st=2):
    x = np.asarray(x, np.float32)
    e = np.asarray(knn_edge_index)
    in_dim, N = x.shape
    src, dst = e[0].astype(np.int64), e[1].astype(np.int64)
    meta = build_meta(N, NC, dst, src, tiles_per_st)
    npc, NPAD = meta["npc"], meta["NPAD"]
    GC = (in_dim + P - 1) // P
    GPAD = GC * P

    bf = ml_dtypes.bfloat16
    w0l = np.zeros((GPAD, P), bf); w0l[:in_dim] = np.asarray(W_l0).T.astype(bf)
    w0r = np.zeros((GPAD, P), bf); w0r[:in_dim] = np.asarray(W_r0).T.astype(bf)
    shared = {
        "W0lT": w0l, "W0rT": w0r,
        "W1lT": np.ascontiguousarray(np.asarray(W_l1).T.astype(bf)),
        "W1rT": np.ascontiguousarray(np.asarray(W_r1).T.astype(bf)),
        "b0col": np.asarray(b_l0, np.float32).reshape(P, 1),
        "b1col": np.asarray(b_l1, np.float32).reshape(P, 1),
        "iota": np.broadcast_to(np.arange(P, dtype=np.float32),
                                (P, P)).astype(bf),
    }
    in_maps = []
    for c in range(NC):
        xp = np.zeros((GPAD, NPAD), np.float32)
        xp[:in_dim, :npc] = x[:, c * npc:(c + 1) * npc]
        # swizzle into per-st SBUF layout: [P, sum_s GC*w_s] bf16
        xt = xp.reshape(GC, P, NPAD).transpose(1, 0, 2)  # [P, GC, NPAD]
        blocks = []
        for ts in meta["st_tiles"]:
            w = len(ts) * P
            c0 = ts[0] * P
            blocks.append(xt[:, :, c0:c0 + w].reshape(P, GC * w))
        xsw = np.concatenate(blocks, axis=1).astype(bf)
        m = dict(shared)
        m["x_sw"] = np.ascontiguousarray(xsw)
        m["invt"] = np.ascontiguousarray(
            np.broadcast_to(meta["inv"][c], (P, NPAD)).astype(bf))
        m["idx16"] = np.ascontiguousarray(meta["idx_slab"][c])
        m["dstid"] = np.ascontiguousarray(meta["dstid_slab"][c].astype(bf))
        in_maps.append(m)
    return meta, in_dim, in_maps


def run(inputs, NC=8, tiles_per_st=2, trace=False, **run_kwargs):
    meta, in_dim, in_maps = _prepare(**inputs, NC=NC, tiles_per_st=tiles_per_st)
    nc = build_kernel(meta, in_dim, NC)
    res = bass_utils.run_bass_kernel_spmd(
        nc, in_maps, core_ids=list(range(NC)), trace=trace, **run_kwargs)
    npc = meta["npc"]
    out = np.concatenate(
        [res.results[c]["outT"][:, :npc].T.astype(np.float32)
         for c in range(NC)], axis=0)
    return np.ascontiguousarray(out), res


def kernel(**inputs) -> np.ndarray:
    out, _ = run(inputs)
    return out


# revision 24
# speedup vs baseline: 1.0544x; 1.0544x over previous
"""Trainium2 Bass kernel for nn_CellEncoder (2-layer GraphSAGE, mean aggregation).

Strategy (8 NeuronCores, SPMD, node-partitioned):
  - Core c owns nodes [c*npc, (c+1)*npc).  Aggregation is linear, so the
    dense transform is applied FIRST: z = h @ W_l.T reduces gather width
    from in_dim (1000) to emb (128) per edge.
  - All PE-path data is bf16 (x, weights, z tables, gathered rows, one-hot
    S): bf16 matmuls run 4x faster than fp32 on TRN2 PE and halve the
    AllGather + gather traffic.  PSUM accumulation stays fp32.
  - Per layer: each core computes z for its own nodes, contributes two
    half-slabs to two AllGathers forming table_lo/table_hi (each
    NC*npc/2 rows < 32768 so rows are addressable by int16 dma_gather
    indices).  The lo AllGather is issued mid-loop as soon as the lo
    tiles are produced, overlapping the collective with compute.
  - Edges are grouped by dst tile (128 dsts); each tile's edges are packed
    into chunks of 128 slots (lo-table chunks then hi-table chunks).
    dma_gather (round-robin over all 4 SWDGE queues) pulls the slot rows
    into SBUF; one-hot matrices S[e,d] = (dst(e)==d) are generated on DVE
    in ONE batched is_equal per st-group (3-d broadcast APs, bf16 2x
    mode) and drive the PE accumulation aggT[f,d] += G_chunk.T @ S_chunk
    in PSUM.  Lo-chunk matmuls are issued before hi-chunk matmuls so the
    PE starts as soon as the lo gather lands.
  - Epilogue is feature-major: x = aggT*inv + rb; elu via
    relu(x) [Scalar engine] + exp(min(x,0)) [Scalar] - 1, bf16 on DVE.
  - Output written feature-major bf16 [128, NPAD]; host transposes/trims.

kernel(**inputs) takes FULL inputs, shards internally, runs one NEFF on
cores 0-7 via bass_utils.run_bass_kernel_spmd, returns the full output.
"""
import os
import sys

import numpy as np

for _p in ("/opt/trn_rl_repo", "/root/.axon_site/_ro/trn_rl_repo"):
    if os.path.isdir(_p) and _p not in sys.path:
        sys.path.append(_p)

import ml_dtypes

import concourse.bass as bass
import concourse.bacc as bacc
import concourse.mybir as mybir
import concourse.tile as tile
from concourse import bass_utils

P = 128
F32 = mybir.dt.float32
BF16 = mybir.dt.bfloat16
AF = mybir.ActivationFunctionType
ALU = mybir.AluOpType

# SWDGE descriptor-ring sizing: ring holds scratch//16 descriptors; one
# dma_gather must fit in its queue's ring.
SCRATCH = 49152
GMAX = 22  # chunks (2816 idxs) per dma_gather; ring holds 3072


def build_meta(N, NC, dst, src, tiles_per_st):
    """Static chunk structure (shared across cores; max-over-core sizes) and
    per-core gather-index / dst-id slabs."""
    npc = N // NC
    half = npc // 2
    TPC = (npc + P - 1) // P
    NPAD = TPC * P
    NST = (TPC + tiles_per_st - 1) // tiles_per_st

    c = dst // npc
    d = (dst - c * npc).astype(np.int64)
    t = d // P
    did = d % P
    sc = src // npc
    sp = src - sc * npc
    tb = (sp >= half).astype(np.int64)
    row = sc * half + np.where(tb == 0, sp, sp - half)
    assert row.max() < 32768

    nlohi = np.zeros((NC, TPC, 2), np.int64)
    np.add.at(nlohi, (c, t, tb), 1)
    KL = np.maximum(1, (nlohi[:, :, 0].max(axis=0) + P - 1) // P)
    KH = ((nlohi[:, :, 1].max(axis=0) + P - 1) // P).astype(np.int64)

    Ktot = KL + KH
    chunk_base = np.concatenate([[0], np.cumsum(Ktot)])
    NCHUNK = int(chunk_base[-1])

    st_tiles = [list(range(s * tiles_per_st, min((s + 1) * tiles_per_st, TPC)))
                for s in range(NST)]
    GL = [int(sum(KL[tt] for tt in ts)) for ts in st_tiles]
    GH = [int(sum(KH[tt] for tt in ts)) for ts in st_tiles]

    idx_off_lo, idx_off_hi = [], []
    off = 0
    for s in range(NST):
        idx_off_lo.append(off); off += GL[s] * P // 16
        idx_off_hi.append(off); off += GH[s] * P // 16
    NIDX16 = off

    idx_slab = np.zeros((NC, P, NIDX16), np.int16)
    dstid_slab = np.full((NC, P, NCHUNK), -1.0, np.float32)
    cnt = np.zeros((NC, NPAD), np.int64)

    order = np.lexsort((tb, t, c))
    co, to, tbo = c[order], t[order], tb[order]
    rowo, dido, do_ = row[order], did[order], d[order]
    np.add.at(cnt, (co, do_), 1)

    key = (co * TPC + to) * 2 + tbo
    bounds = np.concatenate([[0], np.nonzero(np.diff(key))[0] + 1, [len(key)]])
    gval_lo = [np.zeros((NC, GL[s] * P), np.int16) for s in range(NST)]
    gval_hi = [np.zeros((NC, GH[s] * P), np.int16) for s in range(NST)]

    lo_base = np.zeros(TPC, np.int64)
    hi_base = np.zeros(TPC, np.int64)
    for s, ts in enumerate(st_tiles):
        accl = acch = 0
        for tt in ts:
            lo_base[tt] = accl; accl += KL[tt] * P
            hi_base[tt] = acch; acch += KH[tt] * P

    for bi in range(len(bounds) - 1):
        lo_, hi_ = int(bounds[bi]), int(bounds[bi + 1])
        if lo_ == hi_:
            continue
        cc, tt, bb = int(co[lo_]), int(to[lo_]), int(tbo[lo_])
        n = hi_ - lo_
        s = tt // tiles_per_st
        if bb == 0:
            base = int(lo_base[tt])
            gval_lo[s][cc, base:base + n] = rowo[lo_:hi_]
            ch0 = int(chunk_base[tt])
        else:
            base = int(hi_base[tt])
            gval_hi[s][cc, base:base + n] = rowo[lo_:hi_]
            ch0 = int(chunk_base[tt]) + int(KL[tt])
        # base is a multiple of P: slot partition (base+i)%P == i%P and
        # gather block base//P + i//P lines up with tile chunk ch0 + i//P.
        local = np.arange(n)
        dstid_slab[cc, local % P, ch0 + local // P] = dido[lo_:hi_]

    for s in range(NST):
        for cc in range(NC):
            for vals, o in ((gval_lo[s][cc], idx_off_lo[s]),
                            (gval_hi[s][cc], idx_off_hi[s])):
                n = len(vals)
                if n == 0:
                    continue
                w = vals.reshape(n // 16, 16).T
                idx_slab[cc, :, o:o + n // 16] = np.tile(w, (8, 1))

    inv = (1.0 / np.maximum(cnt, 1)).astype(np.float32)

    return dict(
        npc=npc, half=half, TPC=TPC, NPAD=NPAD, NST=NST, st_tiles=st_tiles,
        KL=[int(v) for v in KL], KH=[int(v) for v in KH],
        chunk_base=[int(v) for v in chunk_base], NCHUNK=NCHUNK,
        GL=GL, GH=GH, idx_off_lo=idx_off_lo, idx_off_hi=idx_off_hi,
        NIDX16=NIDX16, idx_slab=idx_slab, dstid_slab=dstid_slab, inv=inv,
    )


# ---------------------------------------------------------------------------
# device kernel builder
# ---------------------------------------------------------------------------

def build_kernel(meta, in_dim, NC):
    npc, half = meta["npc"], meta["half"]
    TPC, NPAD, NST = meta["TPC"], meta["NPAD"], meta["NST"]
    NCHUNK, NIDX16 = meta["NCHUNK"], meta["NIDX16"]
    KL, KH, chunk_base = meta["KL"], meta["KH"], meta["chunk_base"]
    GC = (in_dim + P - 1) // P
    GPAD = GC * P
    XTOT = sum(GC * len(ts) * P for ts in meta["st_tiles"])
    gq = [0]  # gather queue round-robin over 0..3
    # st index after which the lo half (rows < half) is fully produced
    lo_done_st = next(s for s, ts in enumerate(meta["st_tiles"])
                      if (ts[-1] + 1) * P >= half)

    nc = bacc.Bacc("TRN2", target_bir_lowering=False, debug=False,
                   enable_asserts=False, num_devices=NC,
                   dynamic_dma_scratch_size=SCRATCH, num_swdge_queues=4)

    x_d = nc.dram_tensor("x_sw", [P, XTOT], BF16, kind="ExternalInput").ap()
    w0l_d = nc.dram_tensor("W0lT", [GPAD, P], BF16, kind="ExternalInput").ap()
    w0r_d = nc.dram_tensor("W0rT", [GPAD, P], BF16, kind="ExternalInput").ap()
    w1l_d = nc.dram_tensor("W1lT", [P, P], BF16, kind="ExternalInput").ap()
    w1r_d = nc.dram_tensor("W1rT", [P, P], BF16, kind="ExternalInput").ap()
    b0_d = nc.dram_tensor("b0col", [P, 1], F32, kind="ExternalInput").ap()
    b1_d = nc.dram_tensor("b1col", [P, 1], F32, kind="ExternalInput").ap()
    inv_d = nc.dram_tensor("invt", [P, NPAD], BF16, kind="ExternalInput").ap()
    idx_d = nc.dram_tensor("idx16", [P, NIDX16], mybir.dt.int16,
                           kind="ExternalInput").ap()
    iota_d = nc.dram_tensor("iota", [P, P], BF16, kind="ExternalInput").ap()
    dst_d = nc.dram_tensor("dstid", [P, NCHUNK], BF16, kind="ExternalInput").ap()
    out_d = nc.dram_tensor("outT", [P, NPAD], BF16, kind="ExternalOutput").ap()

    with tile.TileContext(nc, num_cores=NC) as tc:
        with (
            tc.tile_pool(name="const", bufs=1) as cpool,
            tc.tile_pool(name="slab", bufs=1) as slab,
            tc.tile_pool(name="xp", bufs=3) as xpool,
            tc.tile_pool(name="gat", bufs=6) as gpool,
            tc.tile_pool(name="sp", bufs=2) as spool,
            tc.tile_pool(name="zp", bufs=3) as zpool,
            tc.tile_pool(name="ep", bufs=2) as epool,
            tc.tile_pool(name="pz", bufs=2, space="PSUM") as pz,
            tc.tile_pool(name="pr", bufs=2, space="PSUM") as pr,
            tc.tile_pool(name="pa", bufs=2, space="PSUM") as pa,
            tc.tile_pool(name="dram", bufs=1, space="DRAM") as dram,
        ):
            # ---- constants ----
            w0l_sb = cpool.tile([P, GC * P], BF16)
            w0r_sb = cpool.tile([P, GC * P], BF16)
            for gc in range(GC):
                nc.sync.dma_start(out=w0l_sb[:, gc * P:(gc + 1) * P],
                                  in_=w0l_d[gc * P:(gc + 1) * P, :])
                nc.sync.dma_start(out=w0r_sb[:, gc * P:(gc + 1) * P],
                                  in_=w0r_d[gc * P:(gc + 1) * P, :])
            w1l_sb = cpool.tile([P, P], BF16)
            nc.sync.dma_start(out=w1l_sb[:], in_=w1l_d[:])
            w1r_sb = cpool.tile([P, P], BF16)
            nc.sync.dma_start(out=w1r_sb[:], in_=w1r_d[:])
            b0_sb = cpool.tile([P, 1], F32)
            nc.sync.dma_start(out=b0_sb[:], in_=b0_d[:])
            b1_sb = cpool.tile([P, 1], F32)
            nc.sync.dma_start(out=b1_sb[:], in_=b1_d[:])
            zero_sb = cpool.tile([P, 1], BF16)
            nc.vector.memset(zero_sb[:], 0.0)
            mone_sb = cpool.tile([P, 1], BF16)
            nc.vector.memset(mone_sb[:], -1.0)
            iota_sb = cpool.tile([P, P], BF16)
            nc.sync.dma_start(out=iota_sb[:], in_=iota_d[:])
            dst_sb = cpool.tile([P, NCHUNK], BF16)
            nc.sync.dma_start(out=dst_sb[:], in_=dst_d[:])
            inv_sb = cpool.tile([P, NPAD], BF16)
            nc.sync.dma_start(out=inv_sb[:], in_=inv_d[:])
            idx_sb = cpool.tile([P, NIDX16], mybir.dt.int16)
            nc.sync.dma_start(out=idx_sb[:], in_=idx_d[:])
            GW = max(meta["GL"][s] + meta["GH"][s] for s in range(NST))
            GHALF = max(meta["GL"] + meta["GH"])

            rb0_sb = slab.tile([P, NPAD], BF16)
            rb1_sb = slab.tile([P, NPAD], BF16)

            # ---- collective buffers ----
            def cc_pair(nm):
                i_lo = dram.tile([half, P], BF16, name=f"cci_lo{nm}")
                i_hi = dram.tile([half, P], BF16, name=f"cci_hi{nm}")
                o_lo = dram.tile([NC * half, P], BF16, addr_space="Shared",
                                 name=f"cco_lo{nm}")
                o_hi = dram.tile([NC * half, P], BF16, addr_space="Shared",
                                 name=f"cco_hi{nm}")
                return i_lo, i_hi, o_lo, o_hi

            cc0 = cc_pair("0")
            cc1 = cc_pair("1")
            rg = [list(range(NC))]

            def z_to_cc(z_sb, tt, cc):
                # scalar-engine HWDGE ring: keeps these cast-gated writes off
                # the sync ring so x loads / idx preloads stream freely
                r0, r1 = tt * P, min(tt * P + P, npc)
                for lo_s, hi_s, tgt, base in (
                        (r0, min(r1, half), cc[0], 0),
                        (max(r0, half), r1, cc[1], half)):
                    if hi_s > lo_s:
                        nc.scalar.dma_start(
                            out=tgt[lo_s - base:hi_s - base, :],
                            in_=z_sb[lo_s - r0:hi_s - r0, :])

            def ag(cc, which):
                nc.gpsimd.collective_compute(
                    "AllGather", ALU.bypass, replica_groups=rg,
                    ins=[cc[which][:].opt()], outs=[cc[which + 2][:].opt()])

            # ---- phase A: z0 (node-major) + rb0T (feature-major) ----
            xoff = 0
            for s, ts in enumerate(meta["st_tiles"]):
                w = len(ts) * P
                c0 = ts[0] * P
                xg = xpool.tile([P, GC * w], BF16, tag="xg",
                                padded_shape=[P, GC * 2 * P])
                nc.sync.dma_start(out=xg[:], in_=x_d[:, xoff:xoff + GC * w])
                xoff += GC * w
                r0ps = pr.tile([P, w], F32, tag="rps", padded_shape=[P, 2 * P])
                for gc in range(GC):
                    nc.tensor.matmul(out=r0ps[:],
                                     lhsT=w0r_sb[:, gc * P:(gc + 1) * P],
                                     rhs=xg[:, gc * w:(gc + 1) * w],
                                     start=(gc == 0), stop=(gc == GC - 1))
                nc.vector.tensor_tensor(out=rb0_sb[:, c0:c0 + w], in0=r0ps[:],
                                        in1=b0_sb[:, :1].to_broadcast([P, w]),
                                        op=ALU.add)
                for ti, tt in enumerate(ts):
                    z0ps = pz.tile([P, P], F32, tag="zps")
                    for gc in range(GC):
                        nc.tensor.matmul(
                            out=z0ps[:],
                            lhsT=xg[:, gc * w + ti * P:gc * w + (ti + 1) * P],
                            rhs=w0l_sb[:, gc * P:(gc + 1) * P],
                            start=(gc == 0), stop=(gc == GC - 1))
                    z0sb = zpool.tile([P, P], BF16, tag="zsb")
                    nc.vector.tensor_copy(out=z0sb[:], in_=z0ps[:])
                    z_to_cc(z0sb, tt, cc0)
                if s == lo_done_st:
                    ag(cc0, 0)
            ag(cc0, 1)

            def gather_split(table, nch, idx_sb, idx_off, tag, bufs=3):
                """One or more dma_gathers (<= GMAX chunks each) into one
                SBUF tile [P, nch*P]."""
                if nch == 0:
                    return None
                g = gpool.tile([P, nch * P], BF16, tag=tag, bufs=bufs,
                               padded_shape=[P, GHALF * P])
                done = 0
                while done < nch:
                    n = min(GMAX, nch - done)
                    gq[0] = (gq[0] + 1) % 4
                    nc.gpsimd.dma_gather(
                        out_ap=g[:, done * P:(done + n) * P]
                        .rearrange("p (k e) -> p k e", e=P),
                        in_ap=table[:],
                        idxs_ap=idx_sb[:, idx_off + done * 8:
                                       idx_off + (done + n) * 8],
                        num_idxs=n * P, num_idxs_reg=n * P, elem_size=P,
                        single_packet=(n * P <= 1024), queue_num=gq[0])
                    done += n
                return g

            # ---- aggregation + epilogue (shared for both layers) ----
            # ghi lags glo by LAG st-groups so AG-hi waiters never exhaust the
            # GpSimd wait queue (which would stall glo issue at layer start)
            LAG = 5

            def aggregate(s, ts, glo, tables, rb_slab, out_cb, post_cb=None):
                w = len(ts) * P
                c0 = ts[0] * P
                GLs, GHs = meta["GL"][s], meta["GH"][s]
                nch_st = GLs + GHs
                o_lo = meta["idx_off_lo"][s]
                ghi = gather_split(tables[1], GHs, idx_sb, o_lo + GLs * 8, "ghi",
                                   bufs=5)
                # batched one-hot S for all chunks of this st-group:
                # S[p, j, d] = (dstid[p, chunk j] == d), bf16
                cb0 = chunk_base[ts[0]]
                s_all = spool.tile([P, nch_st * P], BF16, tag="ssb",
                                   padded_shape=[P, GW * P])
                nc.vector.tensor_tensor(
                    out=s_all[:].rearrange("p (n d) -> p n d", d=P),
                    in0=dst_sb[:, cb0:cb0 + nch_st].to_broadcast([P, nch_st, P]),
                    in1=iota_sb[:].rearrange("p (one d) -> p one d", one=1)
                    .to_broadcast([P, nch_st, P]),
                    op=ALU.is_equal)

                def s_col(cg):
                    j = cg - cb0
                    return s_all[:, j * P:(j + 1) * P]

                # separate PSUM tiles per dst tile so lo-first ordering keeps
                # at most one open accumulation group per PSUM bank
                aggps = [pa.tile([P, P], F32, tag=f"aggps{ti}",
                                 name=f"aggps{ti}") for ti in range(len(ts))]
                # lo chunks first (start accumulation), then hi chunks (stop)
                lo_blk = 0
                for ti, tt in enumerate(ts):
                    for j in range(KL[tt]):
                        g_ap = glo[:, (lo_blk + j) * P:(lo_blk + j + 1) * P]
                        nc.tensor.matmul(out=aggps[ti][:],
                                         lhsT=g_ap, rhs=s_col(chunk_base[tt] + j),
                                         start=(j == 0), stop=False)
                    lo_blk += KL[tt]
                hi_blk = 0
                for ti, tt in enumerate(ts):
                    for j in range(KH[tt]):
                        g_ap = ghi[:, (hi_blk + j) * P:(hi_blk + j + 1) * P]
                        nc.tensor.matmul(out=aggps[ti][:],
                                         lhsT=g_ap,
                                         rhs=s_col(chunk_base[tt] + KL[tt] + j),
                                         start=False, stop=(j == KH[tt] - 1))
                    hi_blk += KH[tt]
                # epilogue: x = aggT*inv + rb ; h = relu(x) + exp(min(x,0)) - 1
                x2 = epool.tile([P, w], BF16, tag="x2", padded_shape=[P, 2 * P])
                for ti, tt in enumerate(ts):
                    nc.vector.tensor_tensor(
                        out=x2[:, ti * P:(ti + 1) * P], in0=aggps[ti][:],
                        in1=inv_sb[:, tt * P:(tt + 1) * P], op=ALU.mult)
                x3 = epool.tile([P, w], BF16, tag="x3", padded_shape=[P, 2 * P])
                nc.vector.tensor_tensor(out=x3[:], in0=x2[:],
                                        in1=rb_slab[:, c0:c0 + w], op=ALU.add)
                xm = epool.tile([P, w], BF16, tag="xm", padded_shape=[P, 2 * P])
                nc.scalar.activation(out=xm[:], in_=x3[:], func=AF.Relu)
                xc = epool.tile([P, w], BF16, tag="xc", padded_shape=[P, 2 * P])
                nc.vector.tensor_tensor(out=xc[:], in0=x3[:],
                                        in1=zero_sb[:, :1].to_broadcast([P, w]),
                                        op=ALU.min)
                xe = epool.tile([P, w], BF16, tag="xe", padded_shape=[P, 2 * P])
                nc.scalar.activation(out=xe[:], in_=xc[:], func=AF.Exp)
                h = epool.tile([P, w], BF16, tag="h", padded_shape=[P, 2 * P])
                nc.vector.scalar_tensor_tensor(out=h[:], in0=xm[:], scalar=-1.0,
                                               in1=xe[:], op0=ALU.add,
                                               op1=ALU.add)
                out_cb(s, ts, w, c0, h)
                if post_cb is not None:
                    post_cb()

            # ---- phase B+C: layer-0 aggregate -> h1T -> z1/rb1T ----
            def l0_out(s, ts, w, c0, h):
                for ti, tt in enumerate(ts):
                    z1ps = pz.tile([P, P], F32, tag="zps")
                    nc.tensor.matmul(out=z1ps[:],
                                     lhsT=h[:, ti * P:(ti + 1) * P],
                                     rhs=w1l_sb[:], start=True, stop=True)
                    z1sb = zpool.tile([P, P], BF16, tag="zsb")
                    nc.vector.tensor_copy(out=z1sb[:], in_=z1ps[:])
                    z_to_cc(z1sb, tt, cc1)
                r1ps = pr.tile([P, w], F32, tag="rps", padded_shape=[P, 2 * P])
                nc.tensor.matmul(out=r1ps[:], lhsT=w1r_sb[:], rhs=h[:],
                                 start=True, stop=True)
                nc.vector.tensor_tensor(out=rb1_sb[:, c0:c0 + w], in0=r1ps[:],
                                        in1=b1_sb[:, :1].to_broadcast([P, w]),
                                        op=ALU.add)

            def run_layer(tables, rb_slab, out_cb, post_idx=None, post_cb=None):
                glos = {}
                for s in range(NST + LAG):
                    if s < NST:
                        glos[s] = gather_split(
                            tables[0], meta["GL"][s], idx_sb,
                            meta["idx_off_lo"][s], "glo", bufs=LAG + 3)
                    if s >= LAG:
                        t = s - LAG
                        aggregate(t, meta["st_tiles"][t], glos.pop(t), tables,
                                  rb_slab, out_cb,
                                  post_cb=post_cb if t == post_idx else None)

            run_layer((cc0[2], cc0[3]), rb0_sb, l0_out,
                      post_idx=lo_done_st, post_cb=lambda: ag(cc1, 0))
            ag(cc1, 1)

            # ---- phase D: layer-1 aggregate -> output ----
            def l1_out(s, ts, w, c0, h):
                nc.sync.dma_start(out=out_d[:, c0:c0 + w], in_=h[:])

            run_layer((cc1[2], cc1[3]), rb1_sb, l1_out)

    nc.compile()
    return nc


# ---------------------------------------------------------------------------
# entry point
# ---------------------------------------------------------------------------

def _prepare(x, knn_edge_index, W_l0, b_l0, W_r0, W_l1, b_l1, W_r1,
             NC=8, tiles_per_st=2):
    x = np.asarray(x, np.float32)
    e = np.asarray(knn_edge_index)
    in_dim, N = x.shape
    src, dst = e[0].astype(np.int64), e[1].astype(np.int64)
    meta = build_meta(N, NC, dst, src, tiles_per_st)
    npc, NPAD = meta["npc"], meta["NPAD"]
    GC = (in_dim + P - 1) // P
    GPAD = GC * P

    bf = ml_dtypes.bfloat16
    w0l = np.zeros((GPAD, P), bf); w0l[:in_dim] = np.asarray(W_l0).T.astype(bf)
    w0r = np.zeros((GPAD, P), bf); w0r[:in_dim] = np.asarray(W_r0).T.astype(bf)
    shared = {
        "W0lT": w0l, "W0rT": w0r,
        "W1lT": np.ascontiguousarray(np.asarray(W_l1).T.astype(bf)),
        "W1rT": np.ascontiguousarray(np.asarray(W_r1).T.astype(bf)),
        "b0col": np.asarray(b_l0, np.float32).reshape(P, 1),
        "b1col": np.asarray(b_l1, np.float32).reshape(P, 1),
        "iota": np.broadcast_to(np.arange(P, dtype=np.float32),
                                (P, P)).astype(bf),
    }
    in_maps = []
    for c in range(NC):
        xp = np.zeros((GPAD, NPAD), np.float32)
        xp[:in_dim, :npc] = x[:, c * npc:(c + 1) * npc]
        # swizzle into per-st SBUF layout: [P, sum_s GC*w_s] bf16
        xt = xp.reshape(GC, P, NPAD).transpose(1, 0, 2)  # [P, GC, NPAD]
        blocks = []
        for ts in meta["st_tiles"]:
            w = len(ts) * P
            c0 = ts[0] * P
            blocks.append(xt[:, :, c0:c0 + w].reshape(P, GC * w))
        xsw = np.concatenate(blocks, axis=1).astype(bf)
        m = dict(shared)
        m["x_sw"] = np.ascontiguousarray(xsw)
        m["invt"] = np.ascontiguousarray(
            np.broadcast_to(meta["inv"][c], (P, NPAD)).astype(bf))
        m["idx16"] = np.ascontiguousarray(meta["idx_slab"][c])
        m["dstid"] = np.ascontiguousarray(meta["dstid_slab"][c].astype(bf))
        in_maps.append(m)
    return meta, in_dim, in_maps


def run(inputs, NC=8, tiles_per_st=2, trace=False, **run_kwargs):
    meta, in_dim, in_maps = _prepare(**inputs, NC=NC, tiles_per_st=tiles_per_st)
    nc = build_kernel(meta, in_dim, NC)
    res = bass_utils.run_bass_kernel_spmd(
        nc, in_maps, core_ids=list(range(NC)), trace=trace, **run_kwargs)
    npc = meta["npc"]
    out = np.concatenate(
        [res.results[c]["outT"][:, :npc].T.astype(np.float32)
         for c in range(NC)], axis=0)
    return np.ascontiguousarray(out), res


def kernel(**inputs) -> np.ndarray:
    out, _ = run(inputs)
    return out
